# revision 1
# baseline (speedup 1.0000x reference)
"""Trainium2 Bass kernel: ComplexGabor1D layer.

reference math (fp32):
    lin = x @ W.T + b                      # [N, 256]
    env = exp(-3600 * lin^2)
    out = stack([env*cos(30*lin), env*sin(30*lin)], -1)   # [N, 256, 2]

Strategy (8 NeuronCores, data parallel over N):
  * Host: transpose each x shard to [256, N_SH] so the contraction dim (i)
    lands on SBUF partitions with fully-contiguous DMA loads; replicate
    W.T ([in, out]) and the bias (pre-broadcast to 128 partitions).
  * Device, per 1024-row "pair" (2 x 512-row halves, 8 x 128-row chunks):
    fp32r matmuls (x.T tiles stationary, W.T moving) accumulate lin into
    PSUM; a fused DVE scalar_tensor_tensor drains PSUM to SBUF while adding
    the bias (lin_sb = lin*1 + b) so the PE is never gated on ACT phases;
    ACT writes sin/cos straight into the interleaved output tile (real at
    even, imag at odd offsets); the envelope is squared+exp'ed in place on
    lin_sb; DVE multiplies the envelope into both strided halves in place;
    2 MiB output DMA per pair via SWDGE so stores don't block input loads.
  * ACT activation tables: sin and exp live in different table sets
    (~2.7us per switch), so pairs are processed in groups: all trig work
    for a group first, then all envelope work -> 2 switches per group. The
    ACT instruction order is pinned via dep edges to stop the scheduler
    interleaving exp into the sin stream.  A fraction of the squares runs
    on DVE (emitted first, their exps last) to balance ACT vs DVE.
  * cos(t) is computed as sin(t + pi/2).  The argument exceeds the Sin
    LUT's [-pi, pi] window only where |30*lin| > pi/2, i.e. where the
    Gaussian envelope is < 5.2e-5, so the hardware clamp there is
    numerically invisible at the output (abs err <= ~1e-4 of absmax 1.0).
"""

import math

import numpy as np

import concourse.bacc as bacc
import concourse.mybir as mybir
import concourse.tile as tile
from concourse.bass_utils import run_bass_kernel_spmd

N_TOTAL = 262144
IN_F = 256
OUT_F = 256
N_CORES = 8
N_SH = N_TOTAL // N_CORES  # 32768 rows per core

CHUNK = 128  # rows per matmul (PSUM partition dim)
CH_PER_HALF = 4  # chunks per half-pair -> 512 rows
HALVES = 2  # halves per pair -> 1024 rows, F=2048 elementwise ops
GROUP_PAIRS = 5  # pairs per ACT-table-set group

OMEGA = 30.0
NEG_SCALE2 = -3600.0  # -(60^2)

F32 = mybir.dt.float32
F32R = mybir.dt.float32r

_BUILD_CACHE = {}


def _build(n_sh, group_pairs):
    """Build the single-core Bass program (SPMD across cores via in_maps)."""
    key = (n_sh, group_pairs)
    if key in _BUILD_CACHE:
        return _BUILD_CACHE[key]

    rows_per_half = CHUNK * CH_PER_HALF
    rows_per_pair = rows_per_half * HALVES
    assert n_sh % rows_per_pair == 0
    n_pairs = n_sh // rows_per_pair

    nc = bacc.Bacc("TRN2", target_bir_lowering=False, debug=False)

    xt = nc.dram_tensor("xt", [IN_F, n_sh], F32R, kind="ExternalInput").ap()
    wt = nc.dram_tensor("wt", [IN_F, OUT_F], F32R, kind="ExternalInput").ap()
    bias = nc.dram_tensor(
        "bias", [CHUNK, CH_PER_HALF * OUT_F], F32, kind="ExternalInput"
    ).ap()
    out = nc.dram_tensor("out", [n_sh, 2 * OUT_F], F32, kind="ExternalOutput").ap()

    # [i, n] -> [p, ci, n] with i = ci*128 + p
    xt_r = xt.rearrange("(ci p) n -> p ci n", p=CHUNK)
    wt_r = wt.rearrange("(ci p) o -> p ci o", p=CHUNK)
    # row n = pr*1024 + t*512 + c2*256 + 2p + e -> per-partition 4 KiB runs
    out_r = out.rearrange(
        "(pr t c2 p e) f -> pr p t c2 e f", e=2, p=CHUNK, c2=2, t=HALVES
    )

    with tile.TileContext(nc) as tc:
        with (
            tc.tile_pool(name="consts", bufs=1) as consts,
            tc.tile_pool(name="xt", bufs=5) as xt_pool,
            tc.tile_pool(name="linsb", bufs=group_pairs + 1) as linsb_pool,
            tc.tile_pool(name="outp", bufs=group_pairs + 1) as out_pool,
            tc.tile_pool(name="lin", bufs=4, space="PSUM") as psum_pool,
        ):
            wt_sb = consts.tile([CHUNK, IN_F // CHUNK, OUT_F], F32R)
            nc.sync.dma_start(wt_sb[:], wt_r[:])
            b_sb = consts.tile([CHUNK, CH_PER_HALF, OUT_F], F32)
            nc.sync.dma_start(
                b_sb[:], bias.rearrange("p (c o) -> p c o", c=CH_PER_HALF)
            )
            zero_b = consts.tile([CHUNK, 1], F32)
            nc.vector.memset(zero_b[:], 0.0)
            pio2_b = consts.tile([CHUNK, 1], F32)
            nc.vector.memset(pio2_b[:], math.pi / 2)

            prev_act = [None]

            def act_chain(inst):
                # Pin the ACT engine's instruction order to emission order so
                # the scheduler cannot interleave exp into the sin stream
                # (each such jump costs two ~1.3us ACT table loads).
                if prev_act[0] is not None:
                    tile.add_dep_helper(inst.ins, prev_act[0], sync=False,
                                        reason="act table-set order")
                prev_act[0] = inst.ins

            n_groups = (n_pairs + group_pairs - 1) // group_pairs
            for g in range(n_groups):
                pairs = range(g * group_pairs, min((g + 1) * group_pairs, n_pairs))
                staged = []

                # ---- trig phase (sin table set resident) ----
                for pr in pairs:
                    n0 = pr * rows_per_pair
                    # one 1 MiB load covering the pair: 4 KiB runs/partition
                    xt_t = xt_pool.tile([CHUNK, IN_F // CHUNK, rows_per_pair], F32R)
                    nc.sync.dma_start(xt_t[:], xt_r[:, :, n0 : n0 + rows_per_pair])

                    lin_sb = linsb_pool.tile(
                        [CHUNK, HALVES, CH_PER_HALF, OUT_F], F32
                    )
                    # row j*2+e view of the pair's columns, for row-pairing
                    xt_v = xt_t[:].rearrange("p ci (j e) -> p ci j e", e=2)
                    for t in range(HALVES):
                        lin = psum_pool.tile([CHUNK, CH_PER_HALF, OUT_F], F32)
                        for c2 in range(2):
                            for e in range(2):
                                j0 = t * (rows_per_half // 2) + c2 * CHUNK
                                lhsT0 = xt_v[:, 0, j0 : j0 + CHUNK, e]
                                lhsT1 = xt_v[:, 1, j0 : j0 + CHUNK, e]
                                c = c2 * 2 + e
                                nc.tensor.matmul(
                                    lin[:, c, :],
                                    lhsT0,
                                    wt_sb[:, 0, :],
                                    start=True,
                                    stop=False,
                                )
                                nc.tensor.matmul(
                                    lin[:, c, :],
                                    lhsT1,
                                    wt_sb[:, 1, :],
                                    start=False,
                                    stop=True,
                                )
                        # drain PSUM with a fused bias add: lin_sb = lin + b
                        nc.vector.scalar_tensor_tensor(
                            lin_sb[:, t, :, :],
                            lin[:],
                            1.0,
                            b_sb[:],
                            op0=mybir.AluOpType.mult,
                            op1=mybir.AluOpType.add,
                        )

                    out_t = out_pool.tile(
                        [CHUNK, HALVES, CH_PER_HALF, 2 * OUT_F], F32
                    )
                    out5 = out_t[:].rearrange(
                        "p t c (o two) -> p t c o two", two=2
                    )
                    # imag = sin(30*lin), real = cos = sin(30*lin + pi/2)
                    act_chain(nc.scalar.activation(
                        out5[:, :, :, :, 1],
                        lin_sb[:],
                        mybir.ActivationFunctionType.Sin,
                        bias=zero_b[:],
                        scale=OMEGA,
                    ))
                    act_chain(nc.scalar.activation(
                        out5[:, :, :, :, 0],
                        lin_sb[:],
                        mybir.ActivationFunctionType.Sin,
                        bias=pio2_b[:],
                        scale=OMEGA,
                    ))
                    staged.append((pr, out_t, lin_sb))

                # ---- envelope phase (exp table set resident) ----
                # ~30% of squares go to DVE: emitted first, their exps last,
                # so ACT never waits on a just-in-time DVE square.
                dve_sq = [s for s in staged if s[0] % 10 in (2, 5, 8)]
                act_sq = [s for s in staged if s[0] % 10 not in (2, 5, 8)]
                for pr, out_t, env in dve_sq:
                    nc.vector.tensor_mul(env[:], env[:], env[:])
                for with_act_square, group_part in ((True, act_sq), (False, dve_sq)):
                    for pr, out_t, env in group_part:
                        if with_act_square:
                            act_chain(nc.scalar.activation(
                                env[:],
                                env[:],
                                mybir.ActivationFunctionType.Square,
                                bias=zero_b[:],
                                scale=1.0,
                            ))
                        act_chain(nc.scalar.activation(
                            env[:],
                            env[:],
                            mybir.ActivationFunctionType.Exp,
                            bias=zero_b[:],
                            scale=NEG_SCALE2,
                        ))
                        out5 = out_t[:].rearrange(
                            "p t c (o two) -> p t c o two", two=2
                        )
                        nc.vector.tensor_mul(
                            out5[:, :, :, :, 0], out5[:, :, :, :, 0], env[:]
                        )
                        nc.vector.tensor_mul(
                            out5[:, :, :, :, 1], out5[:, :, :, :, 1], env[:]
                        )
                        # SWDGE so output stores don't head-of-line block loads
                        nc.gpsimd.dma_start(out_r[pr], out_t[:])

    nc.compile()
    _BUILD_CACHE[key] = nc
    return nc


def run_sharded(x, W, b, trace=False, n_sh=N_SH, group_pairs=GROUP_PAIRS):
    """Shard inputs over the 8 cores, run the Bass kernel, gather output."""
    x = np.ascontiguousarray(x, dtype=np.float32)
    W = np.ascontiguousarray(W, dtype=np.float32)
    b = np.ascontiguousarray(b, dtype=np.float32)
    n = x.shape[0]
    assert n == n_sh * N_CORES and x.shape[1] == IN_F

    nc = _build(n_sh, group_pairs)

    wt_np = np.ascontiguousarray(W.T)  # [in, out]
    b_np = np.ascontiguousarray(
        np.broadcast_to(
            np.tile(b, CH_PER_HALF)[None, :], (CHUNK, CH_PER_HALF * OUT_F)
        )
    )
    in_maps = []
    for s in range(N_CORES):
        xt_np = np.ascontiguousarray(x[s * n_sh : (s + 1) * n_sh].T)  # [in, n_sh]
        in_maps.append({"xt": xt_np, "wt": wt_np, "bias": b_np})

    res = run_bass_kernel_spmd(nc, in_maps, list(range(N_CORES)), trace=trace)
    shards = [
        res.results[s]["out"].reshape(n_sh, OUT_F, 2) for s in range(N_CORES)
    ]
    return np.concatenate(shards, axis=0), res


def kernel(x, W, b):
    out, _ = run_sharded(x, W, b)
    return out



# revision 2
# speedup vs baseline: 1.4387x; 1.4387x over previous
"""Trainium2 Bass kernel: ComplexGabor1D layer.

reference math (fp32):
    lin = x @ W.T + b                      # [N, 256]
    env = exp(-3600 * lin^2)
    out = stack([env*cos(30*lin), env*sin(30*lin)], -1)   # [N, 256, 2]

Strategy (8 NeuronCores, data parallel over N):
  * Host: transpose each x shard to [256, N_SH] bf16 so the contraction dim
    lands on SBUF partitions with contiguous DMA loads; replicate W.T (bf16)
    and the bias (pre-broadcast fp32). bf16 inputs halve the input HBM
    traffic and double PE matmul rate; the resulting |dlin| ~ 3e-5 is far
    inside the 2e-2 output tolerance.
  * Device, per 2048-row block: bf16 matmuls accumulate lin into PSUM fp32;
    a DVE scalar_tensor_tensor drains PSUM to a bf16 lin tile while adding
    the bias. ACT then runs exactly three passes per element:
      imag' = sin(30*lin)          (Sin table)
      real' = sin(30*lin + pi/2)   (= cos, same table)
      env'  = Derivative_Erf(60*lin) = 2/sqrt(pi) * exp(-3600*lin^2)
    Derivative_Erf IS the Gabor envelope up to the 2/sqrt(pi) factor, so no
    Square/Exp passes are needed. DVE folds sqrt(pi)/2 into env with a 4x
    tensor_scalar, then multiplies env into both planes with 2x bf16
    tensor_tensor ops.
  * Output is written PLANAR bf16 ([block, p, pair, plane, chunk, out] with
    16 KiB contiguous per partition per block -> near-ideal store DMA,
    half the bytes of fp32); the host de-interleaves and upcasts to fp32.
    bf16 output rounding (~2e-3) is well inside tolerance.
  * sin and derivative_erf live in different ACT table sets (~2.7us per
    switch), so blocks are processed in groups: all trig for a group, then
    all envelope -> 2 switches per group. The ACT instruction order is
    pinned via dep edges. The matmul+drain work of group g+1 is emitted
    between trig(g) and env(g) so the in-order DVE stream issues the next
    group's PSUM drains before this group's envelope multiplies (keeps PE
    fed through the table-switch phases).
  * sin table is accurate to |x| ~ 4 (measured); our max |arg| is ~3.3 and
    the envelope there is < 1e-8, so no range reduction is needed.
"""

import math

import numpy as np
from ml_dtypes import bfloat16

import concourse.bacc as bacc
import concourse.mybir as mybir
import concourse.tile as tile
from concourse.bass_utils import run_bass_kernel_spmd

N_TOTAL = 262144
IN_F = 256
OUT_F = 256
N_CORES = 8
N_SH = N_TOTAL // N_CORES  # 32768 rows per core

CHUNK = 128    # rows per matmul (PSUM partition dim)
HALF = 1024    # rows per PSUM tile (8 chunks)
BLOCK = 2048   # rows per ACT/DVE superblock (FD=4096 per instruction)
GROUP_BLOCKS = 4  # blocks per ACT-table-set group

OMEGA = 30.0
DERF_SCALE = 60.0           # Derivative_Erf(60*lin) = 2/sqrt(pi)*exp(-3600*lin^2)
SQRTPI_2 = math.sqrt(math.pi) / 2

F32 = mybir.dt.float32
BF16 = mybir.dt.bfloat16

_BUILD_CACHE = {}


def _build(n_sh, group_blocks):
    """Build the single-core Bass program (SPMD across cores via in_maps)."""
    key = (n_sh, group_blocks)
    if key in _BUILD_CACHE:
        return _BUILD_CACHE[key]

    assert n_sh % BLOCK == 0
    n_blocks = n_sh // BLOCK
    cph = HALF // CHUNK  # chunks per PSUM tile (8)

    nc = bacc.Bacc("TRN2", target_bir_lowering=False, debug=False)

    xt = nc.dram_tensor("xt", [IN_F, n_sh], BF16, kind="ExternalInput").ap()
    wt = nc.dram_tensor("wt", [IN_F, OUT_F], BF16, kind="ExternalInput").ap()
    bias = nc.dram_tensor("bias", [CHUNK, cph * OUT_F], F32, kind="ExternalInput").ap()
    # row n = blk*2048 + h*1024 + c*128 + p ; plane e in {real, imag}
    out = nc.dram_tensor(
        "out", [n_blocks, CHUNK, 2, 2, cph, OUT_F], BF16, kind="ExternalOutput"
    ).ap()

    # [i, n] -> [p, ci, n] with i = ci*128 + p
    xt_r = xt.rearrange("(ci p) n -> p ci n", p=CHUNK)
    wt_r = wt.rearrange("(ci p) o -> p ci o", p=CHUNK)

    with tile.TileContext(nc) as tc:
        with (
            tc.tile_pool(name="consts", bufs=1) as consts,
            tc.tile_pool(name="xt", bufs=4) as xt_pool,
            tc.tile_pool(name="lin", bufs=2 * group_blocks + 1) as lin_pool,
            tc.tile_pool(name="outp", bufs=group_blocks + 1) as out_pool,
            tc.tile_pool(name="ps", bufs=2, space="PSUM") as psum_pool,
        ):
            wt_sb = consts.tile([CHUNK, 2, OUT_F], BF16)
            nc.sync.dma_start(wt_sb[:], wt_r[:])
            b_sb = consts.tile([CHUNK, cph, OUT_F], F32)
            nc.sync.dma_start(b_sb[:], bias.rearrange("p (c o) -> p c o", c=cph))
            zero_b = consts.tile([CHUNK, 1], F32)
            nc.vector.memset(zero_b[:], 0.0)
            pio2_b = consts.tile([CHUNK, 1], F32)
            nc.vector.memset(pio2_b[:], math.pi / 2)

            prev_act = [None]

            def act_chain(inst):
                # Pin the ACT engine's instruction order to emission order so
                # the scheduler cannot interleave derivative_erf into the sin
                # stream (each jump costs two ~1.3us ACT table loads).
                if prev_act[0] is not None:
                    tile.add_dep_helper(inst.ins, prev_act[0], sync=False,
                                        reason="act table-set order")
                prev_act[0] = inst.ins

            lin_tiles = {}
            out_tiles = {}

            def phase_a(blk):
                # load + matmul + PSUM drain (bias add) for one block
                n0 = blk * BLOCK
                xt_t = xt_pool.tile([CHUNK, 2, BLOCK], BF16)
                nc.sync.dma_start(xt_t[:], xt_r[:, :, n0 : n0 + BLOCK])
                lin_sb = lin_pool.tile([CHUNK, 2, cph, OUT_F], BF16)
                for h in range(2):
                    ps = psum_pool.tile([CHUNK, cph, OUT_F], F32)
                    for c in range(cph):
                        r0 = h * HALF + c * CHUNK
                        for ci in range(2):
                            nc.tensor.matmul(
                                ps[:, c, :],
                                xt_t[:, ci, r0 : r0 + CHUNK],
                                wt_sb[:, ci, :],
                                start=(ci == 0),
                                stop=(ci == 1),
                            )
                    # drain PSUM with a fused bias add: lin_sb = lin + b (bf16)
                    nc.vector.scalar_tensor_tensor(
                        lin_sb[:, h],
                        ps[:],
                        1.0,
                        b_sb[:],
                        op0=mybir.AluOpType.mult,
                        op1=mybir.AluOpType.add,
                    )
                lin_tiles[blk] = lin_sb

            groups = [
                list(range(g, min(g + group_blocks, n_blocks)))
                for g in range(0, n_blocks, group_blocks)
            ]

            for blk in groups[0]:
                phase_a(blk)

            for gi, grp in enumerate(groups):
                # ---- trig phase (sin table set resident) ----
                for blk in grp:
                    lin_sb = lin_tiles[blk]
                    out_t = out_pool.tile([CHUNK, 2, 2, cph, OUT_F], BF16)
                    out_tiles[blk] = out_t
                    act_chain(nc.scalar.activation(
                        out_t[:, :, 1],
                        lin_sb[:],
                        mybir.ActivationFunctionType.Sin,
                        bias=zero_b[:],
                        scale=OMEGA,
                    ))
                    act_chain(nc.scalar.activation(
                        out_t[:, :, 0],
                        lin_sb[:],
                        mybir.ActivationFunctionType.Sin,
                        bias=pio2_b[:],
                        scale=OMEGA,
                    ))

                # next group's matmuls+drains, emitted here so the DVE issues
                # them before this group's envelope multiplies
                if gi + 1 < len(groups):
                    for blk in groups[gi + 1]:
                        phase_a(blk)

                # ---- envelope phase (erf_derivative table set resident) ----
                for blk in grp:
                    lin_sb = lin_tiles.pop(blk)
                    out_t = out_tiles.pop(blk)
                    act_chain(nc.scalar.activation(
                        lin_sb[:],
                        lin_sb[:],
                        mybir.ActivationFunctionType.Derivative_Erf,
                        bias=zero_b[:],
                        scale=DERF_SCALE,
                    ))
                    nc.vector.tensor_scalar_mul(lin_sb[:], lin_sb[:], SQRTPI_2)
                    nc.vector.tensor_mul(out_t[:, :, 0], out_t[:, :, 0], lin_sb[:])
                    nc.vector.tensor_mul(out_t[:, :, 1], out_t[:, :, 1], lin_sb[:])
                    # SWDGE so output stores don't head-of-line block loads
                    nc.gpsimd.dma_start(out[blk], out_t[:])

    nc.compile()
    _BUILD_CACHE[key] = nc
    return nc


def run_sharded(x, W, b, trace=False, n_sh=N_SH, group_blocks=GROUP_BLOCKS):
    """Shard inputs over the 8 cores, run the Bass kernel, gather output."""
    x = np.asarray(x, dtype=np.float32)
    W = np.asarray(W, dtype=np.float32)
    b = np.asarray(b, dtype=np.float32)
    n = x.shape[0]
    assert n == n_sh * N_CORES and x.shape[1] == IN_F

    nc = _build(n_sh, group_blocks)

    cph = HALF // CHUNK
    wt_np = np.ascontiguousarray(W.T).astype(bfloat16)  # [in, out]
    b_np = np.ascontiguousarray(
        np.broadcast_to(np.tile(b, cph)[None, :], (CHUNK, cph * OUT_F))
    )
    in_maps = []
    for s in range(N_CORES):
        xt_np = np.ascontiguousarray(
            x[s * n_sh : (s + 1) * n_sh].T.astype(bfloat16)
        )  # [in, n_sh] bf16
        in_maps.append({"xt": xt_np, "wt": wt_np, "bias": b_np})

    res = run_bass_kernel_spmd(nc, in_maps, list(range(N_CORES)), trace=trace)

    n_blocks = n_sh // BLOCK
    shards = []
    for s in range(N_CORES):
        arr = np.asarray(res.results[s]["out"])  # [blk, p, h, e, c, o] bf16
        arr = arr.reshape(n_blocks, CHUNK, 2, 2, cph, OUT_F)
        # row n = blk*2048 + h*1024 + c*128 + p ; want [n, o, e] fp32
        full = arr.transpose(0, 2, 4, 1, 5, 3).reshape(n_sh, OUT_F, 2)
        shards.append(full.astype(np.float32))
    return np.concatenate(shards, axis=0), res


def kernel(x, W, b):
    out, _ = run_sharded(x, W, b)
    return out


# revision 3
# speedup vs baseline: 1.4521x; 1.0093x over previous
"""Trainium2 Bass kernel: ComplexGabor1D layer.

reference math (fp32):
    lin = x @ W.T + b                      # [N, 256]
    env = exp(-3600 * lin^2)
    out = stack([env*cos(30*lin), env*sin(30*lin)], -1)   # [N, 256, 2]

Strategy (8 NeuronCores, data parallel over N):
  * Host: transpose each x shard to [256, N_SH] bf16 so the contraction dim
    lands on SBUF partitions with contiguous DMA loads; replicate W.T (bf16)
    and the bias (pre-broadcast fp32). bf16 inputs halve the input HBM
    traffic and double PE matmul rate; the resulting |dlin| ~ 3e-5 is far
    inside the 2e-2 output tolerance.
  * Device, per 2048-row block: bf16 matmuls accumulate lin into PSUM fp32;
    a DVE scalar_tensor_tensor drains PSUM to a bf16 lin tile while adding
    the bias. ACT then runs exactly three passes per element:
      imag' = sin(30*lin)          (Sin table)
      real' = sin(30*lin + pi/2)   (= cos, same table)
      env'  = Derivative_Erf(60*lin) = 2/sqrt(pi) * exp(-3600*lin^2)
    Derivative_Erf IS the Gabor envelope up to the 2/sqrt(pi) factor, so no
    Square/Exp passes are needed. DVE folds sqrt(pi)/2 into env with a 4x
    tensor_scalar, then multiplies env into both planes with 2x bf16
    tensor_tensor ops. ACT is the bottleneck engine at ~85% busy; its three
    passes are the floor (no table set fuses trig with a gaussian, and DVE
    polynomial substitutes cost ~3x what they save).
  * Output is written PLANAR bf16 ([block, p, half, plane, chunk, out], one
    DMA per plane with 4 KiB runs); the host de-interleaves and upcasts to
    fp32. bf16 output rounding (~2e-3) is well inside tolerance.
  * sin and derivative_erf live in different ACT table sets (~2.6us per
    switch = load + pipeline drain), so blocks are processed in groups
    ([6,5,5] for 16 blocks): all trig for a group, then all envelope -> 2
    switches per group, 6 loads total. A dummy sin at program start pulls
    the first table load into the pipeline-fill window. The ACT instruction
    order is pinned via dep edges.
  * The matmul+drain work of group g+1 is software-pipelined: its first
    block is emitted between trig(g) and env(g), the rest interleaved into
    env(g), so the in-order DVE stream issues the next group's PSUM drains
    before/between this group's envelope multiplies and the ACT never waits
    on a drain at a group boundary. Block 0's trig is emitted per half so
    the first sin starts after half a block's worth of DMA+matmul+drain.
  * sin table is accurate to |x| ~ 4 (measured); our max |arg| is ~3.3 and
    the envelope there is < 1e-8, so no range reduction is needed.
"""

import math

import numpy as np
from ml_dtypes import bfloat16

import concourse.bacc as bacc
import concourse.mybir as mybir
import concourse.tile as tile
from concourse.bass_utils import run_bass_kernel_spmd

N_TOTAL = 262144
IN_F = 256
OUT_F = 256
N_CORES = 8
N_SH = N_TOTAL // N_CORES  # 32768 rows per core

CHUNK = 128    # rows per matmul (PSUM partition dim)
HALF = 1024    # rows per PSUM tile (8 chunks)
BLOCK = 2048   # rows per ACT/DVE superblock (FD=4096 per instruction)
N_GROUPS = 3   # ACT-table-set groups (2 table switches per group)

OMEGA = 30.0
DERF_SCALE = 60.0           # Derivative_Erf(60*lin) = 2/sqrt(pi)*exp(-3600*lin^2)
SQRTPI_2 = math.sqrt(math.pi) / 2

F32 = mybir.dt.float32
BF16 = mybir.dt.bfloat16

_BUILD_CACHE = {}


def _build(n_sh, n_groups):
    """Build the single-core Bass program (SPMD across cores via in_maps)."""
    key = (n_sh, n_groups)
    if key in _BUILD_CACHE:
        return _BUILD_CACHE[key]

    assert n_sh % BLOCK == 0
    n_blocks = n_sh // BLOCK
    cph = HALF // CHUNK  # chunks per PSUM tile (8)

    # group sizes, as equal as possible, larger first: e.g. 16 -> [6, 5, 5]
    base, rem = divmod(n_blocks, n_groups)
    sizes = [base + (1 if i < rem else 0) for i in range(n_groups)]
    groups, pos = [], 0
    for sz in sizes:
        groups.append(list(range(pos, pos + sz)))
        pos += sz

    nc = bacc.Bacc("TRN2", target_bir_lowering=False, debug=False)

    xt = nc.dram_tensor("xt", [IN_F, n_sh], BF16, kind="ExternalInput").ap()
    wt = nc.dram_tensor("wt", [IN_F, OUT_F], BF16, kind="ExternalInput").ap()
    bias = nc.dram_tensor("bias", [CHUNK, cph * OUT_F], F32, kind="ExternalInput").ap()
    # row n = blk*2048 + h*1024 + c*128 + p ; plane e in {real, imag}
    out = nc.dram_tensor(
        "out", [n_blocks, CHUNK, 2, 2, cph, OUT_F], BF16, kind="ExternalOutput"
    ).ap()

    # [i, n] -> [p, ci, n] with i = ci*128 + p
    xt_r = xt.rearrange("(ci p) n -> p ci n", p=CHUNK)
    wt_r = wt.rearrange("(ci p) o -> p ci o", p=CHUNK)

    with tile.TileContext(nc) as tc:
        with (
            tc.tile_pool(name="consts", bufs=1) as consts,
            tc.tile_pool(name="xt", bufs=4) as xt_pool,
            tc.tile_pool(name="lin", bufs=9) as lin_pool,
            tc.tile_pool(name="outp", bufs=6) as out_pool,
            tc.tile_pool(name="ps", bufs=2, space="PSUM") as psum_pool,
        ):
            wt_sb = consts.tile([CHUNK, 2, OUT_F], BF16)
            nc.sync.dma_start(wt_sb[:], wt_r[:])
            b_sb = consts.tile([CHUNK, cph, OUT_F], F32)
            nc.sync.dma_start(b_sb[:], bias.rearrange("p (c o) -> p c o", c=cph))
            zero_b = consts.tile([CHUNK, 1], F32)
            nc.vector.memset(zero_b[:], 0.0)
            pio2_b = consts.tile([CHUNK, 1], F32)
            nc.vector.memset(pio2_b[:], math.pi / 2)

            prev_act = [None]

            def act_chain(inst):
                # Pin the ACT engine's instruction order to emission order so
                # the scheduler cannot interleave derivative_erf into the sin
                # stream (each jump costs two ~1.3us ACT table loads).
                if prev_act[0] is not None:
                    tile.add_dep_helper(inst.ins, prev_act[0], sync=False,
                                        reason="act table-set order")
                prev_act[0] = inst.ins

            # dummy sin: pulls the first Sin table load into the pipeline-fill
            # window so the first real trig instruction doesn't pay it
            warm = consts.tile([CHUNK, 1], BF16)
            act_chain(nc.scalar.activation(
                warm[:], zero_b[:], mybir.ActivationFunctionType.Sin,
                bias=zero_b[:], scale=OMEGA,
            ))

            lin_tiles = {}
            out_tiles = {}

            def phase_a(blk):
                # per half: load xt, matmul into PSUM, drain+bias to bf16 SBUF
                lin_sb = lin_pool.tile([CHUNK, 2, cph, OUT_F], BF16)
                for h in range(2):
                    n0 = blk * BLOCK + h * HALF
                    xt_t = xt_pool.tile([CHUNK, 2, HALF], BF16)
                    nc.sync.dma_start(xt_t[:], xt_r[:, :, n0 : n0 + HALF])
                    ps = psum_pool.tile([CHUNK, cph, OUT_F], F32)
                    for c in range(cph):
                        r0 = c * CHUNK
                        for ci in range(2):
                            nc.tensor.matmul(
                                ps[:, c, :],
                                xt_t[:, ci, r0 : r0 + CHUNK],
                                wt_sb[:, ci, :],
                                start=(ci == 0),
                                stop=(ci == 1),
                            )
                    # drain PSUM with a fused bias add: lin_sb = lin + b (bf16)
                    nc.vector.scalar_tensor_tensor(
                        lin_sb[:, h],
                        ps[:],
                        1.0,
                        b_sb[:],
                        op0=mybir.AluOpType.mult,
                        op1=mybir.AluOpType.add,
                    )
                lin_tiles[blk] = lin_sb

            def trig(blk, per_half):
                lin_sb = lin_tiles[blk]
                out_t = out_pool.tile([CHUNK, 2, 2, cph, OUT_F], BF16)
                out_tiles[blk] = out_t
                halves = [(h,) for h in range(2)] if per_half else [(slice(None),)]
                for (h,) in halves:
                    act_chain(nc.scalar.activation(
                        out_t[:, h, 1],
                        lin_sb[:, h],
                        mybir.ActivationFunctionType.Sin,
                        bias=zero_b[:],
                        scale=OMEGA,
                    ))
                    act_chain(nc.scalar.activation(
                        out_t[:, h, 0],
                        lin_sb[:, h],
                        mybir.ActivationFunctionType.Sin,
                        bias=pio2_b[:],
                        scale=OMEGA,
                    ))

            def env(blk):
                lin_sb = lin_tiles.pop(blk)
                out_t = out_tiles.pop(blk)
                act_chain(nc.scalar.activation(
                    lin_sb[:],
                    lin_sb[:],
                    mybir.ActivationFunctionType.Derivative_Erf,
                    bias=zero_b[:],
                    scale=DERF_SCALE,
                ))
                nc.vector.tensor_scalar_mul(lin_sb[:], lin_sb[:], SQRTPI_2)
                for e in range(2):
                    nc.vector.tensor_mul(out_t[:, :, e], out_t[:, :, e], lin_sb[:])
                    # SWDGE so output stores don't head-of-line block loads;
                    # per plane so the store starts right after its multiply
                    nc.gpsimd.dma_start(out[blk][:, :, e], out_t[:, :, e])

            for blk in groups[0]:
                phase_a(blk)

            for gi, grp in enumerate(groups):
                nxt = groups[gi + 1] if gi + 1 < len(groups) else []
                # ---- trig phase (sin table set resident) ----
                for k, blk in enumerate(grp):
                    trig(blk, per_half=(gi == 0 and k == 0))
                # next group's first block, emitted here so its PSUM drain
                # precedes this group's multiplies in the in-order DVE stream
                if nxt:
                    phase_a(nxt[0])
                # ---- envelope phase (erf_derivative table set resident) ----
                for j, blk in enumerate(grp):
                    env(blk)
                    if j + 1 < len(nxt):
                        phase_a(nxt[j + 1])

    nc.compile()
    _BUILD_CACHE[key] = nc
    return nc


def run_sharded(x, W, b, trace=False, n_sh=N_SH, n_groups=N_GROUPS):
    """Shard inputs over the 8 cores, run the Bass kernel, gather output."""
    x = np.asarray(x, dtype=np.float32)
    W = np.asarray(W, dtype=np.float32)
    b = np.asarray(b, dtype=np.float32)
    n = x.shape[0]
    assert n == n_sh * N_CORES and x.shape[1] == IN_F

    nc = _build(n_sh, n_groups)

    cph = HALF // CHUNK
    wt_np = np.ascontiguousarray(W.T).astype(bfloat16)  # [in, out]
    b_np = np.ascontiguousarray(
        np.broadcast_to(np.tile(b, cph)[None, :], (CHUNK, cph * OUT_F))
    )
    in_maps = []
    for s in range(N_CORES):
        xt_np = np.ascontiguousarray(
            x[s * n_sh : (s + 1) * n_sh].T.astype(bfloat16)
        )  # [in, n_sh] bf16
        in_maps.append({"xt": xt_np, "wt": wt_np, "bias": b_np})

    res = run_bass_kernel_spmd(nc, in_maps, list(range(N_CORES)), trace=trace)

    n_blocks = n_sh // BLOCK
    shards = []
    for s in range(N_CORES):
        arr = np.asarray(res.results[s]["out"])  # [blk, p, h, e, c, o] bf16
        arr = arr.reshape(n_blocks, CHUNK, 2, 2, cph, OUT_F)
        # row n = blk*2048 + h*1024 + c*128 + p ; want [n, o, e] fp32
        full = arr.transpose(0, 2, 4, 1, 5, 3).reshape(n_sh, OUT_F, 2)
        shards.append(full.astype(np.float32))
    return np.concatenate(shards, axis=0), res


def kernel(x, W, b):
    out, _ = run_sharded(x, W, b)
    return out


# revision 6
# speedup vs baseline: 1.5051x; 1.0365x over previous
"""Trainium2 Bass kernel: ComplexGabor1D layer.

reference math (fp32):
    lin = x @ W.T + b                      # [N, 256]
    env = exp(-3600 * lin^2)
    out = stack([env*cos(30*lin), env*sin(30*lin)], -1)   # [N, 256, 2]

Strategy (8 NeuronCores, data parallel over N):
  * Host: transpose each x shard to [256, N_SH] bf16 so the contraction dim
    lands on SBUF partitions with contiguous DMA loads; replicate W.T (bf16)
    and the bias (pre-broadcast fp32). bf16 inputs halve the input HBM
    traffic and double PE matmul rate; the resulting |dlin| ~ 3e-5 is far
    inside the 2e-2 output tolerance.
  * Device, per 2048-row block: bf16 matmuls accumulate lin into PSUM fp32;
    a DVE scalar_tensor_tensor drains PSUM to a bf16 lin tile while adding
    the bias. ACT then runs exactly three passes per element:
      imag' = sin(30*lin)          (Sin table)
      real' = sin(30*lin + pi/2)   (= cos, same table)
      env'  = Derivative_Erf(60*lin) = 2/sqrt(pi) * exp(-3600*lin^2)
    Derivative_Erf IS the Gabor envelope up to the 2/sqrt(pi) factor, so no
    Square/Exp passes are needed. DVE folds sqrt(pi)/2 into env with a 4x
    tensor_scalar, then multiplies env into both planes with 2x bf16
    tensor_tensor ops. ACT is the bottleneck engine at ~85% busy; its three
    passes are the floor (no table set fuses trig with a gaussian, and DVE
    polynomial substitutes cost ~3x what they save).
  * Output is written PLANAR bf16 ([block, p, half, plane, chunk, out], one
    DMA per plane with 4 KiB runs); the host de-interleaves and upcasts to
    fp32. bf16 output rounding (~2e-3) is well inside tolerance.
  * sin and derivative_erf live in different ACT table sets (~2.6us per
    switch = load + pipeline drain), so blocks are processed in groups
    ([6,5,5] for 16 blocks): all trig for a group, then all envelope -> 2
    switches per group, 6 loads total. A dummy sin at program start pulls
    the first table load into the pipeline-fill window. The ACT instruction
    order is pinned via dep edges.
  * The matmul+drain work of group g+1 is software-pipelined: its first
    block is emitted between trig(g) and env(g), the rest interleaved into
    env(g), so the in-order DVE stream issues the next group's PSUM drains
    before/between this group's envelope multiplies and the ACT never waits
    on a drain at a group boundary. Block 0's trig is emitted per half so
    the first sin starts after half a block's worth of DMA+matmul+drain.
  * sin table is accurate to |x| ~ 4 (measured); our max |arg| is ~3.3 and
    the envelope there is < 1e-8, so no range reduction is needed.
"""

import math

import numpy as np
from ml_dtypes import bfloat16

import concourse.bacc as bacc
import concourse.mybir as mybir
import concourse.tile as tile
from concourse.bass_utils import run_bass_kernel_spmd

N_TOTAL = 262144
IN_F = 256
OUT_F = 256
N_CORES = 8
N_SH = N_TOTAL // N_CORES  # 32768 rows per core

CHUNK = 128    # rows per matmul (PSUM partition dim)
HALF = 1024    # rows per PSUM tile (8 chunks)
BLOCK = 2048   # rows per ACT/DVE superblock (FD=4096 per instruction)
N_GROUPS = 3   # ACT-table-set groups (2 table switches per group)

OMEGA = 30.0
DERF_SCALE = 60.0           # Derivative_Erf(60*lin) = 2/sqrt(pi)*exp(-3600*lin^2)
SQRTPI_2 = math.sqrt(math.pi) / 2

F32 = mybir.dt.float32
BF16 = mybir.dt.bfloat16

_BUILD_CACHE = {}


def _build(n_sh, n_groups):
    """Build the single-core Bass program (SPMD across cores via in_maps)."""
    key = (n_sh, n_groups)
    if key in _BUILD_CACHE:
        return _BUILD_CACHE[key]

    assert n_sh % BLOCK == 0
    n_blocks = n_sh // BLOCK
    cph = HALF // CHUNK  # chunks per PSUM tile (8)

    # group sizes, as equal as possible, larger first: e.g. 16 -> [6, 5, 5]
    base, rem = divmod(n_blocks, n_groups)
    sizes = [base + (1 if i < rem else 0) for i in range(n_groups)]
    groups, pos = [], 0
    for sz in sizes:
        groups.append(list(range(pos, pos + sz)))
        pos += sz

    nc = bacc.Bacc("TRN2", target_bir_lowering=False, debug=False)

    xt = nc.dram_tensor("xt", [IN_F, n_sh], BF16, kind="ExternalInput").ap()
    wt = nc.dram_tensor("wt", [IN_F, OUT_F], BF16, kind="ExternalInput").ap()
    bias = nc.dram_tensor("bias", [CHUNK, cph * OUT_F], F32, kind="ExternalInput").ap()
    # row n = blk*2048 + h*1024 + c*128 + p ; plane e in {real, imag}
    out = nc.dram_tensor(
        "out", [n_blocks, CHUNK, 2, 2, cph, OUT_F], BF16, kind="ExternalOutput"
    ).ap()

    # [i, n] -> [p, ci, n] with i = ci*128 + p
    xt_r = xt.rearrange("(ci p) n -> p ci n", p=CHUNK)
    wt_r = wt.rearrange("(ci p) o -> p ci o", p=CHUNK)

    with tile.TileContext(nc) as tc:
        with (
            tc.tile_pool(name="consts", bufs=1) as consts,
            tc.tile_pool(name="xt", bufs=4) as xt_pool,
            tc.tile_pool(name="lin", bufs=9) as lin_pool,
            tc.tile_pool(name="outp", bufs=6) as out_pool,
            tc.tile_pool(name="ps", bufs=2, space="PSUM") as psum_pool,
        ):
            # consts go through the SWDGE queue so they don't head-of-line
            # block the first xt loads on the sync queue
            wt_sb = consts.tile([CHUNK, 2, OUT_F], BF16)
            nc.gpsimd.dma_start(wt_sb[:], wt_r[:])
            b_sb = consts.tile([CHUNK, cph, OUT_F], F32)
            nc.gpsimd.dma_start(b_sb[:], bias.rearrange("p (c o) -> p c o", c=cph))
            zero_b = consts.tile([CHUNK, 1], F32)
            nc.vector.memset(zero_b[:], 0.0)
            pio2_b = consts.tile([CHUNK, 1], F32)
            nc.vector.memset(pio2_b[:], math.pi / 2)

            prev_act = [None]

            def act_chain(inst):
                # Pin the ACT engine's instruction order to emission order so
                # the scheduler cannot interleave derivative_erf into the sin
                # stream (each jump costs two ~1.3us ACT table loads).
                if prev_act[0] is not None:
                    tile.add_dep_helper(inst.ins, prev_act[0], sync=False,
                                        reason="act table-set order")
                prev_act[0] = inst.ins

            # dummy sin: pulls the first Sin table load into the pipeline-fill
            # window so the first real trig instruction doesn't pay it
            warm = consts.tile([CHUNK, 1], BF16)
            act_chain(nc.scalar.activation(
                warm[:], zero_b[:], mybir.ActivationFunctionType.Sin,
                bias=zero_b[:], scale=OMEGA,
            ))

            lin_tiles = {}
            out_tiles = {}

            def phase_a(blk):
                # per half: load xt, matmul into PSUM, drain+bias to bf16 SBUF
                lin_sb = lin_pool.tile([CHUNK, 2, cph, OUT_F], BF16)
                for h in range(2):
                    n0 = blk * BLOCK + h * HALF
                    xt_t = xt_pool.tile([CHUNK, 2, HALF], BF16)
                    nc.sync.dma_start(xt_t[:], xt_r[:, :, n0 : n0 + HALF])
                    ps = psum_pool.tile([CHUNK, cph, OUT_F], F32)
                    for c in range(cph):
                        r0 = c * CHUNK
                        for ci in range(2):
                            nc.tensor.matmul(
                                ps[:, c, :],
                                xt_t[:, ci, r0 : r0 + CHUNK],
                                wt_sb[:, ci, :],
                                start=(ci == 0),
                                stop=(ci == 1),
                            )
                    # drain PSUM with a fused bias add: lin_sb = lin + b (bf16)
                    nc.vector.scalar_tensor_tensor(
                        lin_sb[:, h],
                        ps[:],
                        1.0,
                        b_sb[:],
                        op0=mybir.AluOpType.mult,
                        op1=mybir.AluOpType.add,
                    )
                lin_tiles[blk] = lin_sb

            def trig(blk, per_half):
                lin_sb = lin_tiles[blk]
                out_t = out_pool.tile([CHUNK, 2, 2, cph, OUT_F], BF16)
                out_tiles[blk] = out_t
                halves = [(h,) for h in range(2)] if per_half else [(slice(None),)]
                for (h,) in halves:
                    act_chain(nc.scalar.activation(
                        out_t[:, h, 1],
                        lin_sb[:, h],
                        mybir.ActivationFunctionType.Sin,
                        bias=zero_b[:],
                        scale=OMEGA,
                    ))
                    act_chain(nc.scalar.activation(
                        out_t[:, h, 0],
                        lin_sb[:, h],
                        mybir.ActivationFunctionType.Sin,
                        bias=pio2_b[:],
                        scale=OMEGA,
                    ))
                # fold sqrt(pi)/2 into the trig planes here (4x tensor_scalar)
                # where the DVE is idle, instead of scaling the envelope in the
                # env phase where the DVE is the backlog
                for e in range(2):
                    nc.vector.tensor_scalar_mul(
                        out_t[:, :, e], out_t[:, :, e], SQRTPI_2
                    )

            def env(blk):
                lin_sb = lin_tiles.pop(blk)
                out_t = out_tiles.pop(blk)
                act_chain(nc.scalar.activation(
                    lin_sb[:],
                    lin_sb[:],
                    mybir.ActivationFunctionType.Derivative_Erf,
                    bias=zero_b[:],
                    scale=DERF_SCALE,
                ))
                for e in range(2):
                    nc.vector.tensor_mul(out_t[:, :, e], out_t[:, :, e], lin_sb[:])
                    # SWDGE so output stores don't head-of-line block loads;
                    # per plane so the store starts right after its multiply
                    nc.gpsimd.dma_start(out[blk][:, :, e], out_t[:, :, e])

            for blk in groups[0]:
                phase_a(blk)

            for gi, grp in enumerate(groups):
                nxt = groups[gi + 1] if gi + 1 < len(groups) else []
                # ---- trig phase (sin table set resident) ----
                for k, blk in enumerate(grp):
                    trig(blk, per_half=(gi == 0 and k == 0))
                # next group's first block, emitted here so its PSUM drain
                # precedes this group's multiplies in the in-order DVE stream
                if nxt:
                    phase_a(nxt[0])
                # ---- envelope phase (erf_derivative table set resident) ----
                for j, blk in enumerate(grp):
                    env(blk)
                    if j + 1 < len(nxt):
                        phase_a(nxt[j + 1])

    nc.compile()
    _BUILD_CACHE[key] = nc
    return nc


def run_sharded(x, W, b, trace=False, n_sh=N_SH, n_groups=N_GROUPS):
    """Shard inputs over the 8 cores, run the Bass kernel, gather output."""
    x = np.asarray(x, dtype=np.float32)
    W = np.asarray(W, dtype=np.float32)
    b = np.asarray(b, dtype=np.float32)
    n = x.shape[0]
    assert n == n_sh * N_CORES and x.shape[1] == IN_F

    nc = _build(n_sh, n_groups)

    cph = HALF // CHUNK
    wt_np = np.ascontiguousarray(W.T).astype(bfloat16)  # [in, out]
    b_np = np.ascontiguousarray(
        np.broadcast_to(np.tile(b, cph)[None, :], (CHUNK, cph * OUT_F))
    )
    in_maps = []
    for s in range(N_CORES):
        xt_np = np.ascontiguousarray(
            x[s * n_sh : (s + 1) * n_sh].T.astype(bfloat16)
        )  # [in, n_sh] bf16
        in_maps.append({"xt": xt_np, "wt": wt_np, "bias": b_np})

    res = run_bass_kernel_spmd(nc, in_maps, list(range(N_CORES)), trace=trace)

    n_blocks = n_sh // BLOCK
    shards = []
    for s in range(N_CORES):
        arr = np.asarray(res.results[s]["out"])  # [blk, p, h, e, c, o] bf16
        arr = arr.reshape(n_blocks, CHUNK, 2, 2, cph, OUT_F)
        # row n = blk*2048 + h*1024 + c*128 + p ; want [n, o, e] fp32
        full = arr.transpose(0, 2, 4, 1, 5, 3).reshape(n_sh, OUT_F, 2)
        shards.append(full.astype(np.float32))
    return np.concatenate(shards, axis=0), res


def kernel(x, W, b):
    out, _ = run_sharded(x, W, b)
    return out
